# revision 17
# baseline (speedup 1.0000x reference)
"""Circular-pad -> unfold(K=7,S=3) -> 896->64->896 MLP -> fold -> crop, on 8 NeuronCores.

Data-parallel: one batch element per core. The unfold/fold are never
materialized: the padded input is phase-split into 3 contiguous fp32r buffers
(columns mod 3), so patch-chunk k of the unfold is a contiguous slice of
phase k%3; stage 1 is 7 PSUM-accumulated matmuls. The fold is 3 phase-streams
(out columns mod 3), each a PSUM accumulation of 2-3 taps with column-shifted
rhs from h. The per-tap bias b2 rides in the matmul via a ones-row appended
to h; h is zero-padded 2 cols per side so boundary taps are masked and every
fp32r matmul keeps an even free dim.
"""

import numpy as np

import concourse.bass as bass  # noqa: F401
import concourse.mybir as mybir
import concourse.tile as tile
from concourse import bacc
from concourse.bass_utils import run_bass_kernel_spmd
from concourse.tile import add_dep_helper

B, C, L = 8, 128, 16384
K, S, PAD, IC = 7, 3, 3, 64
LP = L + 2 * PAD          # 16390
P = (LP - K) // S + 1     # 5462
W = 512                   # p/q tile width (one PSUM bank of fp32)
NCORES = 8
F32 = mybir.dt.float32
F32R = mybir.dt.float32r
CHUNK = 2049              # input DMA chunk (3-aligned, ~1 MiB)
NPH = [5464, 5463, 5463]  # phase buffer widths (phase r holds xp cols 3q+r)


def _ceil_div(a, b):
    return (a + b - 1) // b


def _body(tc, out, x, w1k, b1, w2k, hones, hzeros):
    nc = tc.nc
    n1 = _ceil_div(P, W)
    qlim = [P + 1, P, P]  # valid q (exclusive) per output stream

    with (
        tc.tile_pool(name="const", bufs=1) as cpool,
        tc.tile_pool(name="big", bufs=1) as bigpool,
        tc.tile_pool(name="stg", bufs=3) as stg,
        tc.tile_pool(name="ps1", bufs=2, space="PSUM") as ps1,
        tc.tile_pool(name="ps2", bufs=4, space="PSUM") as ps2,
        tc.tile_pool(name="psw", bufs=1, space="PSUM") as psw_pool,
    ):
        # --- constants (host-arranged); fp32r via casting DMA ---
        w1t = cpool.tile([C, K * IC], F32R)      # [c, k*64+o] = W1[o, c*7+k]
        nc.gpsimd.dma_start(out=w1t[:], in_=w1k)
        w2t = cpool.tile([IC + 1, K * C], F32R)  # [o, k*128+c] = W2[c*7+k, o]; row 64 = b2
        nc.gpsimd.dma_start(out=w2t[:], in_=w2k)
        b1t = cpool.tile([IC, 1], F32)
        nc.sync.dma_start(out=b1t[:], in_=b1)

        # --- PE warm-up: dummy matmuls on junk data so HAM un-throttles
        # before the real stream arrives (PE would otherwise start cold).
        junk = cpool.tile([C, W], F32)
        nc.vector.memset(junk[:], 0.0)
        psw = psw_pool.tile([IC, W], F32, tag="psw", name="psw")
        for _ in range(6):
            nc.tensor.matmul(psw[:, :], junk[:, 0:IC], junk[:, :],
                             start=True, stop=True)

        # --- padded input (circular), raw fp32 via HWDGE. Wraps issue first
        # (HWDGE is FIFO per engine); chunks are chained so chunk i completes
        # before chunk i+1 starts (early first-tile data instead of
        # round-robin bandwidth sharing across all chunks).
        xf = bigpool.tile([C, LP], F32)
        nc.sync.dma_start(out=xf[:, 0:PAD], in_=x[:, L - PAD:L])
        nc.sync.dma_start(out=xf[:, PAD + L:LP], in_=x[:, 0:PAD])
        prev = None
        for a in range(0, L, CHUNK):
            b = min(a + CHUNK, L)
            dma = nc.sync.dma_start(out=xf[:, PAD + a:PAD + b], in_=x[:, a:b])
            if prev is not None:
                add_dep_helper(dma.ins, prev.ins, sync=True,
                               reason="serialize input chunks")
            prev = dma

        # --- phase-split cast: xph[r][c, q] = xf[c, 3q+r] as fp32r ---
        xph = [bigpool.tile([C, NPH[r]], F32R, tag=f"xph{r}", name=f"xph{r}")
               for r in range(3)]
        # chunk boundaries in xp space, 3-aligned, matching the DMA chunks
        edges = [0] + [PAD + a for a in range(CHUNK - PAD, L, CHUNK)] + [LP]
        for ca, cb in zip(edges[:-1], edges[1:]):
            for r, eng in ((0, nc.vector), (1, nc.scalar), (2, nc.vector)):
                qa, qb = _ceil_div(ca - r, 3), _ceil_div(cb - r, 3)
                src = xf[:, 3 * qa + r: 3 * (qb - 1) + r + 1: 3]
                if eng is nc.scalar:
                    eng.activation(xph[r][:, qa:qb], src,
                                   mybir.ActivationFunctionType.Copy)
                else:
                    eng.tensor_copy(out=xph[r][:, qa:qb], in_=src)

        # --- h (2-col zero pad each side; ones row = stage-2 bias tap) ---
        h = bigpool.tile([IC + 1, P + 4], F32R)
        nc.gpsimd.dma_start(out=h[IC:IC + 1, :], in_=hones)
        nc.gpsimd.dma_start(out=h[0:IC, 0:2], in_=hzeros[:, 0:2])
        nc.gpsimd.dma_start(out=h[0:IC, P + 2:P + 4], in_=hzeros[:, 2:4])

        def stage1(i):
            p0 = i * W
            w = min(W, P - p0)
            ps = ps1.tile([IC, W], F32, tag="ps1")
            for k in range(K):
                rhs = xph[k % 3][:, k // 3 + p0: k // 3 + p0 + w]
                nc.tensor.matmul(
                    ps[:, :w],
                    w1t[:, k * IC:(k + 1) * IC],
                    rhs,
                    start=(k == 0),
                    stop=(k == K - 1),
                )
            nc.scalar.activation(
                h[0:IC, 2 + p0:2 + p0 + w], ps[:, :w],
                mybir.ActivationFunctionType.Relu, bias=b1t[:],
            )

        def stage2(j):
            qa = 1 + j * W
            qb = min(qa + W, P + 1)   # same (even) matmul width for all streams
            w = qb - qa
            blk = stg.tile([C, 3 * W], F32, tag="blk")
            for r, eng in ((0, nc.vector), (1, nc.scalar), (2, nc.vector)):
                ps = ps2.tile([C, W], F32, tag="ps2")
                taps = [k for k in (r, r + 3, r + 6) if k < K]
                for ti, kk in enumerate(taps):
                    d = (kk - r) // 3
                    nc.tensor.matmul(
                        ps[:, :w],
                        w2t[:, kk * C:(kk + 1) * C],
                        h[:, 2 + qa - d: 2 + qb - d],
                        start=(ti == 0),
                        stop=(ti == len(taps) - 1),
                    )
                # evict valid columns with stride-3 interleave into block staging
                we = min(qb, qlim[r]) - qa
                dst = blk[:, r:r + 3 * (we - 1) + 1:3]
                if eng is nc.scalar:
                    eng.activation(dst, ps[:, :we],
                                   mybir.ActivationFunctionType.Copy)
                else:
                    eng.tensor_copy(out=dst, in_=ps[:, :we])
            a = 3 * (qa - 1)
            b = min(a + 3 * W, L)
            nc.sync.dma_start(out=out[:, a:b], in_=blk[:, 0:b - a])

        stage1(0)
        for i in range(1, n1):
            stage1(i)
            stage2(i - 1)
        stage2(n1 - 1)


_CACHE = {}


def _build():
    if "nc" in _CACHE:
        return _CACHE["nc"]
    nc = bacc.Bacc("TRN2", target_bir_lowering=False, debug=False,
                   num_devices=NCORES)
    x = nc.dram_tensor("x", [C, L], F32, kind="ExternalInput").ap()
    w1k = nc.dram_tensor("w1k", [C, K * IC], F32, kind="ExternalInput").ap()
    b1 = nc.dram_tensor("b1", [IC, 1], F32, kind="ExternalInput").ap()
    w2k = nc.dram_tensor("w2k", [IC + 1, K * C], F32, kind="ExternalInput").ap()
    hones = nc.dram_tensor("hones", [1, P + 4], F32, kind="ExternalInput").ap()
    hzeros = nc.dram_tensor("hzeros", [IC, 4], F32, kind="ExternalInput").ap()
    out = nc.dram_tensor("out", [C, L], F32, kind="ExternalOutput").ap()
    with tile.TileContext(nc) as tc:
        _body(tc, out, x, w1k, b1, w2k, hones, hzeros)
    nc.compile()
    _CACHE["nc"] = nc
    return nc


def _prep_weights(W1, b1, W2, b2):
    # w1k[c, k*IC+o] = W1[o, c*7+k]
    w1k = np.ascontiguousarray(
        W1.reshape(IC, C, K).transpose(1, 2, 0).reshape(C, K * IC),
        dtype=np.float32)
    # w2k[o, k*C+c] = W2[c*7+k, o] for o<IC; row IC = b2[c*7+k]
    w2 = W2.reshape(C, K, IC).transpose(2, 1, 0).reshape(IC, K * C)
    b2row = b2.reshape(C, K).T.reshape(1, K * C)
    w2k = np.ascontiguousarray(np.vstack([w2, b2row]), dtype=np.float32)
    return w1k, np.ascontiguousarray(b1.reshape(IC, 1), dtype=np.float32), w2k


def kernel(x, W1, b1, W2, b2, _trace=False):
    nc = _build()
    w1k, b1c, w2k = _prep_weights(
        np.asarray(W1, np.float32), np.asarray(b1, np.float32),
        np.asarray(W2, np.float32), np.asarray(b2, np.float32))
    x = np.asarray(x, np.float32)
    hones = np.zeros((1, P + 4), np.float32)
    hones[0, 2:P + 2] = 1.0
    hzeros = np.zeros((IC, 4), np.float32)
    in_maps = [
        {"x": np.ascontiguousarray(x[i]), "w1k": w1k, "b1": b1c, "w2k": w2k,
         "hones": hones, "hzeros": hzeros}
        for i in range(NCORES)
    ]
    res = run_bass_kernel_spmd(nc, in_maps, core_ids=list(range(NCORES)),
                               trace=_trace)
    out = np.stack([r["out"] for r in res.results], axis=0)
    if _trace:
        kernel.last_results = res
    return out


# revision 26
# speedup vs baseline: 1.1039x; 1.1039x over previous
"""Circular-pad -> unfold(K=7,S=3) -> 896->64->896 MLP -> fold -> crop, on 8 NeuronCores.

Data-parallel: one batch element per core. The unfold/fold are never
materialized: the padded input is phase-split into 3 contiguous fp32r buffers
(columns mod 3), so patch-chunk k of the unfold is a contiguous slice of
phase k%3; stage 1 is 7 PSUM-accumulated matmuls. The fold is 3 phase-streams
(out columns mod 3), each a PSUM accumulation of 2-3 taps with column-shifted
rhs from h. The per-tap bias b2 rides in the matmul via a ones-row appended
to h; h is zero-padded 2 cols per side so boundary taps are masked and every
fp32r matmul keeps an even free dim.
"""

import numpy as np

import concourse.bass as bass  # noqa: F401
import concourse.mybir as mybir
import concourse.tile as tile
from concourse import bacc
from concourse.bass_utils import run_bass_kernel_spmd
from concourse.tile import add_dep_helper

B, C, L = 8, 128, 16384
K, S, PAD, IC = 7, 3, 3, 64
LP = L + 2 * PAD          # 16390
P = (LP - K) // S + 1     # 5462
W = 512                   # p/q tile width (one PSUM bank of fp32)
NCORES = 8
F32 = mybir.dt.float32
F32R = mybir.dt.float32r
NPH = [5464, 5463, 5463]  # phase buffer widths (phase r holds xp cols 3q+r)
# input chunk edges in x columns (3-aligned; chunk 0 covers stage-1 tile 0,
# later chunks ~1023 cols, streamed with a window-3 dependency ladder)
XEDGES = [0, 1542] + list(range(2565, 16384, 1023)) + [16384]


def _ceil_div(a, b):
    return (a + b - 1) // b


def _body(tc, out, x, w1k, b1, w2k, hones, hzeros):
    nc = tc.nc
    n1 = _ceil_div(P, W)
    qlim = [P + 1, P, P]  # valid q (exclusive) per output stream

    with (
        tc.tile_pool(name="const", bufs=1) as cpool,
        tc.tile_pool(name="big", bufs=1) as bigpool,
        tc.tile_pool(name="stg", bufs=3) as stg,
        tc.tile_pool(name="ps1", bufs=2, space="PSUM") as ps1,
        tc.tile_pool(name="ps2", bufs=4, space="PSUM") as ps2,
        tc.tile_pool(name="psw", bufs=1, space="PSUM") as psw_pool,
    ):
        # --- constants (host-arranged); fp32r via casting DMA ---
        w1t = cpool.tile([C, K * IC], F32R)      # [c, k*64+o] = W1[o, c*7+k]
        nc.gpsimd.dma_start(out=w1t[:], in_=w1k)
        w2t = cpool.tile([IC + 1, K * C], F32R)  # [o, k*128+c] = W2[c*7+k, o]; row 64 = b2
        nc.gpsimd.dma_start(out=w2t[:], in_=w2k)
        b1t = cpool.tile([IC, 1], F32)
        nc.sync.dma_start(out=b1t[:], in_=b1)

        # --- PE warm-up: dummy matmuls on junk data so HAM un-throttles
        # before the real stream arrives (PE would otherwise start cold).
        junk = cpool.tile([C, W], F32)
        nc.vector.memset(junk[:], 0.0)
        psw = psw_pool.tile([IC, W], F32, tag="psw", name="psw")
        for _ in range(6):
            nc.tensor.matmul(psw[:, :], junk[:, 0:IC], junk[:, :],
                             start=True, stop=True)

        # --- padded input (circular), raw fp32 via HWDGE. Chunk 0 (exactly
        # what stage-1 tile 0 needs) gets HBM priority: every other chunk
        # waits for its completion, then the rest stream in parallel at
        # line rate.
        xf = bigpool.tile([C, LP], F32)
        nc.sync.dma_start(out=xf[:, 0:PAD], in_=x[:, L - PAD:L])
        nc.sync.dma_start(out=xf[:, PAD + L:LP], in_=x[:, 0:PAD])
        dmas = []
        for a, b in zip(XEDGES[:-1], XEDGES[1:]):
            dma = nc.sync.dma_start(out=xf[:, PAD + a:PAD + b], in_=x[:, a:b])
            k = len(dmas)
            dep = 0 if 1 <= k <= 3 else (k - 3 if k >= 4 else None)
            if dep is not None:
                add_dep_helper(dma.ins, dmas[dep].ins, sync=True,
                               reason="input stream ladder")
            dmas.append(dma)

        # --- phase-split cast: xph[r][c, q] = xf[c, 3q+r] as fp32r ---
        xph = [bigpool.tile([C, NPH[r]], F32R, tag=f"xph{r}", name=f"xph{r}")
               for r in range(3)]
        # copy-chunk boundaries in xp space: pair up DMA chunks (except the
        # first) so the per-op overhead stays low
        xe = XEDGES[:1] + XEDGES[1::2] + ([] if XEDGES[-1] == XEDGES[1::2][-1]
                                          else XEDGES[-1:])
        edges = [0] + [PAD + a for a in xe[1:-1]] + [LP]
        for ca, cb in zip(edges[:-1], edges[1:]):
            for r in range(3):
                qa, qb = _ceil_div(ca - r, 3), _ceil_div(cb - r, 3)
                src = xf[:, 3 * qa + r: 3 * (qb - 1) + r + 1: 3]
                if r == 0:
                    nc.gpsimd.tensor_copy(out=xph[r][:, qa:qb], in_=src)
                elif r == 1:
                    nc.scalar.activation(xph[r][:, qa:qb], src,
                                         mybir.ActivationFunctionType.Copy)
                else:
                    nc.vector.tensor_copy(out=xph[r][:, qa:qb], in_=src)

        # --- h (2-col zero pad each side; ones row = stage-2 bias tap) ---
        h = bigpool.tile([IC + 1, P + 4], F32R)
        nc.gpsimd.dma_start(out=h[IC:IC + 1, :], in_=hones)
        nc.gpsimd.dma_start(out=h[0:IC, 0:2], in_=hzeros[:, 0:2])
        nc.gpsimd.dma_start(out=h[0:IC, P + 2:P + 4], in_=hzeros[:, 2:4])

        def stage1(i):
            p0 = i * W
            w = min(W, P - p0)
            ps = ps1.tile([IC, W], F32, tag="ps1")
            for k in range(K):
                rhs = xph[k % 3][:, k // 3 + p0: k // 3 + p0 + w]
                nc.tensor.matmul(
                    ps[:, :w],
                    w1t[:, k * IC:(k + 1) * IC],
                    rhs,
                    start=(k == 0),
                    stop=(k == K - 1),
                )
            nc.scalar.activation(
                h[0:IC, 2 + p0:2 + p0 + w], ps[:, :w],
                mybir.ActivationFunctionType.Relu, bias=b1t[:],
            )

        def stage2(j):
            qa = 1 + j * W
            qb = min(qa + W, P + 1)   # same (even) matmul width for all streams
            w = qb - qa
            blk = stg.tile([C, 3 * W], F32, tag="blk")
            for r, eng in ((0, nc.vector), (1, nc.scalar), (2, nc.vector)):
                ps = ps2.tile([C, W], F32, tag="ps2")
                taps = [k for k in (r, r + 3, r + 6) if k < K]
                for ti, kk in enumerate(taps):
                    d = (kk - r) // 3
                    nc.tensor.matmul(
                        ps[:, :w],
                        w2t[:, kk * C:(kk + 1) * C],
                        h[:, 2 + qa - d: 2 + qb - d],
                        start=(ti == 0),
                        stop=(ti == len(taps) - 1),
                    )
                # evict valid columns with stride-3 interleave into block staging
                we = min(qb, qlim[r]) - qa
                dst = blk[:, r:r + 3 * (we - 1) + 1:3]
                if eng is nc.scalar:
                    eng.activation(dst, ps[:, :we],
                                   mybir.ActivationFunctionType.Copy)
                else:
                    eng.tensor_copy(out=dst, in_=ps[:, :we])
            a = 3 * (qa - 1)
            b = min(a + 3 * W, L)
            if b == L:
                # last block is on the critical path: split across partition
                # slices so the per-DMA latency is paid in parallel
                for p in range(0, C, 32):
                    nc.sync.dma_start(out=out[p:p + 32, a:b],
                                      in_=blk[p:p + 32, 0:b - a])
            else:
                nc.sync.dma_start(out=out[:, a:b], in_=blk[:, 0:b - a])

        stage1(0)
        for i in range(1, n1):
            stage1(i)
            stage2(i - 1)
        stage2(n1 - 1)


_CACHE = {}


def _build():
    if "nc" in _CACHE:
        return _CACHE["nc"]
    nc = bacc.Bacc("TRN2", target_bir_lowering=False, debug=False,
                   num_devices=NCORES)
    x = nc.dram_tensor("x", [C, L], F32, kind="ExternalInput").ap()
    w1k = nc.dram_tensor("w1k", [C, K * IC], F32, kind="ExternalInput").ap()
    b1 = nc.dram_tensor("b1", [IC, 1], F32, kind="ExternalInput").ap()
    w2k = nc.dram_tensor("w2k", [IC + 1, K * C], F32, kind="ExternalInput").ap()
    hones = nc.dram_tensor("hones", [1, P + 4], F32, kind="ExternalInput").ap()
    hzeros = nc.dram_tensor("hzeros", [IC, 4], F32, kind="ExternalInput").ap()
    out = nc.dram_tensor("out", [C, L], F32, kind="ExternalOutput").ap()
    with tile.TileContext(nc) as tc:
        _body(tc, out, x, w1k, b1, w2k, hones, hzeros)
    nc.compile()
    _CACHE["nc"] = nc
    return nc


def _prep_weights(W1, b1, W2, b2):
    # w1k[c, k*IC+o] = W1[o, c*7+k]
    w1k = np.ascontiguousarray(
        W1.reshape(IC, C, K).transpose(1, 2, 0).reshape(C, K * IC),
        dtype=np.float32)
    # w2k[o, k*C+c] = W2[c*7+k, o] for o<IC; row IC = b2[c*7+k]
    w2 = W2.reshape(C, K, IC).transpose(2, 1, 0).reshape(IC, K * C)
    b2row = b2.reshape(C, K).T.reshape(1, K * C)
    w2k = np.ascontiguousarray(np.vstack([w2, b2row]), dtype=np.float32)
    return w1k, np.ascontiguousarray(b1.reshape(IC, 1), dtype=np.float32), w2k


def kernel(x, W1, b1, W2, b2, _trace=False):
    nc = _build()
    w1k, b1c, w2k = _prep_weights(
        np.asarray(W1, np.float32), np.asarray(b1, np.float32),
        np.asarray(W2, np.float32), np.asarray(b2, np.float32))
    x = np.asarray(x, np.float32)
    hones = np.zeros((1, P + 4), np.float32)
    hones[0, 2:P + 2] = 1.0
    hzeros = np.zeros((IC, 4), np.float32)
    in_maps = [
        {"x": np.ascontiguousarray(x[i]), "w1k": w1k, "b1": b1c, "w2k": w2k,
         "hones": hones, "hzeros": hzeros}
        for i in range(NCORES)
    ]
    res = run_bass_kernel_spmd(nc, in_maps, core_ids=list(range(NCORES)),
                               trace=_trace)
    out = np.stack([r["out"] for r in res.results], axis=0)
    if _trace:
        kernel.last_results = res
    return out
